# revision 1
# baseline (speedup 1.0000x reference)
"""ECGformer forward pass on 8 TRN2 NeuronCores, data-parallel over batch.

Layout strategy per core (1 batch element):
- Residual stream h: token-major fp32, [128, 9*256] (q-tile j at cols 256j;
  tile 8 holds only token 1024 in row 0).
- LayerNorm in token-major (per-partition stats), affine applied during the
  PE-transpose evacuation (feature-major, per-partition scale/bias on ACT).
- Attention in [k, q] feature-major: energy^T = K^T-slice.T @ Q^T via K=32
  row-tiled matmuls; softmax denominator comes free from a 16.0-valued
  augmentation column in V (also folds the post-softmax /sqrt(E) division);
  O^T accumulated per head with M=33 matmuls; normalization via
  reciprocal + gpsimd partition-broadcast + fused multiply-evacuation.
- All matmul operands bf16 (fp32 PSUM accumulation); weights fed bf16 from
  host, permuted so attention head h lives in "slot" s with 32-aligned rows.
- Biases: bq/bk added per-partition during Q^T/K^T evac; bv+bo folded into a
  host-precomputed row added via a K=1 ones-matmul into the Wo PSUM; b1 added
  per-partition in the gelu evac; b2/b_emb/bc1/bc2 via K=1 ones-matmuls.
"""

import os
import sys

import numpy as np

try:
    import ml_dtypes
except ImportError:  # pragma: no cover
    ml_dtypes = None

TRN_REPO = "/opt/trn_rl_repo"

B, N, C = 8, 1024, 12
E, H, HD, L, FF, NCLS = 256, 8, 32, 4, 1024, 5
S = N + 1          # 1025 tokens
NQ = 9             # token tiles (8 full + 1 single-row)
NKT = 9            # k tiles in attention
EPS = 1e-5
SIG = [0, 4, 1, 5, 2, 6, 3, 7]   # slot s holds head SIG[s]
DIV = 16.0         # sqrt(E); folded into the V augmentation column

_CACHE = {}
KPAIR = os.environ.get("KPAIR", "1") == "1"
KSTRAG = os.environ.get("KSTRAG", "0") == "1"
DVEK = {int(v) for v in os.environ.get("DVEK", "0,2,4,6").split(",") if v != ""}


EXPA1, EXPA2, EXPA3 = 0.0625147850914497, 0.0019667051147439383, 4.0039417979884645e-05


def _register_exp_ops():
    import re
    from concourse import dve_ops as D
    from concourse.dve_spec import Spec, Src0, C0, C1, C2, One, sq
    if any(op.name == "ANT_EXPP" for op in D.OPS):
        return
    expp = D.DveOp(
        name="ANT_EXPP",
        spec=Spec(
            body=((C0 * Src0 + C1) * Src0 + C2) * Src0 + One,
            reference=lambda in0, s0, s1, imm2:
                ((s0 * in0 + s1) * in0 + imm2) * in0 + 1.0,
        ),
        subdim=False, uops_sha={})
    sq16 = D.DveOp(
        name="ANT_SQ16",
        spec=Spec(body=sq(sq(sq(sq(Src0)))),
                  reference=lambda in0: (((in0 ** 2) ** 2) ** 2) ** 2),
        subdim=False, uops_sha={})
    for op in (expp, sq16):
        D.OPS.append(op)
        D.CUSTOM_DVE_SPECS[op.name] = op.spec
        D._SUB_OPCODE_FOR_NAME[op.name] = D._CUSTOM_DVE_ROW_BASE + len(D.OPS) - 1
        assert D._SUB_OPCODE_FOR_NAME[op.name] < 0x20
        for ver in ("v3", "v4"):
            try:
                op.compile(ver)
            except ValueError as e:
                m = re.search(r"\(" + ver + r": ([0-9a-f]+)", str(e))
                op.uops_sha[ver] = m.group(1)
        op.compile("v3")
    return expp, sq16


def _build():
    sys.path.insert(0, TRN_REPO)
    import concourse.tile as tile
    from concourse import mybir, bacc
    from concourse.masks import make_identity
    from concourse import dve_ops as D
    _register_exp_ops()
    EXPP = next(op for op in D.OPS if op.name == "ANT_EXPP")
    SQ16 = next(op for op in D.OPS if op.name == "ANT_SQ16")

    F32 = mybir.dt.float32
    BF16 = mybir.dt.bfloat16
    AF = mybir.ActivationFunctionType
    AX = mybir.AxisListType
    OP = mybir.AluOpType

    nc = bacc.Bacc("TRN2", target_bir_lowering=False, debug=False, num_devices=8)

    # ---------------- DRAM parameters ----------------
    def din(name, shape, dt=F32):
        return nc.dram_tensor(name, shape, dt, kind="ExternalInput").ap()

    x_d = din("x", [N, C])
    pos_d = din("pos_tm", [128, NQ * E])
    cls_d = din("cls_row", [1, E])
    wemb_d = din("wemb", [C, E], BF16)
    bemb_d = din("bemb_row", [1, E], BF16)
    gemb_d = din("gemb_bc", [128, E])
    beemb_d = din("beemb_bc", [128, E])

    wq_d, wk_d, wv_d, wo_d, w1_d, w2_d = [], [], [], [], [], []
    ln1s_d, ln1b_d, ln2s_d, ln2b_d = [], [], [], []
    bqf_d, bkf_d, b1f_d, wob_d, b2r_d, wof_d = [], [], [], [], [], []
    for l in range(L):
        wq_d.append(din(f"wq{l}", [128, 2 * E], BF16))
        wk_d.append(din(f"wk{l}", [128, 2 * E], BF16))
        wv_d.append(din(f"wv{l}", [128, 2 * E], BF16))
        wo_d.append(din(f"wo{l}", [96, 8 * E], BF16))
        w1_d.append(din(f"w1{l}", [128, 2 * FF], BF16))
        w2_d.append(din(f"w2{l}", [128, 8 * E], BF16))
        ln1s_d.append(din(f"ln1s{l}", [128, 2]))
        ln1b_d.append(din(f"ln1b{l}", [128, 2]))
        ln2s_d.append(din(f"ln2s{l}", [128, 2]))
        ln2b_d.append(din(f"ln2b{l}", [128, 2]))
        bqf_d.append(din(f"bqf{l}", [128, 2]))
        bkf_d.append(din(f"bkf{l}", [128, 2]))
        b1f_d.append(din(f"b1f{l}", [128, 8]))
        wob_d.append(din(f"wob{l}", [1, E], BF16))
        wof_d.append(din(f"wof{l}", [32, 8 * E], BF16))
        b2r_d.append(din(f"b2r{l}", [1, E], BF16))
    wc1_d = din("wc1", [128, 2 * E], BF16)
    bc1_d = din("bc1r", [1, E], BF16)
    lncg_d = din("lncg", [1, E])
    lncb_d = din("lncb", [1, E])
    wc2_d = din("wc2", [128, 2 * NCLS], BF16)
    bc2_d = din("bc2r", [1, NCLS], BF16)

    out_d = nc.dram_tensor("out", [1, NCLS], F32, kind="ExternalOutput").ap()

    with tile.TileContext(nc) as tc:
        cp = tc.alloc_tile_pool(name="consts", bufs=1)
        ap_ = tc.alloc_tile_pool(name="acts", bufs=1)
        ptp = tc.alloc_tile_pool(name="ptp", bufs=4)
        bcp = tc.alloc_tile_pool(name="bcp", bufs=2)
        dnp = tc.alloc_tile_pool(name="dnp", bufs=1)
        psM = tc.alloc_tile_pool(name="psM", bufs=1, space="PSUM")
        psE = tc.alloc_tile_pool(name="psE", bufs=3, space="PSUM")
        ps2 = tc.alloc_tile_pool(name="ps2", bufs=2, space="PSUM")

        # ---------------- constants into SBUF ----------------
        def load(name, dram, shape, dt=F32):
            t = cp.tile(shape, dt, tag=name)
            nc.sync.dma_start(t[:], dram[:])
            return t

        pos_sb = load("pos", pos_d, [128, NQ * E])
        wemb_sb = load("wemb", wemb_d, [C, E], BF16)
        bemb_sb = load("bemb", bemb_d, [1, E], BF16)
        gemb_sb = load("gemb", gemb_d, [128, E])
        beemb_sb = load("beemb", beemb_d, [128, E])
        wq_sb = [load(f"wq{l}", wq_d[l], [128, 2 * E], BF16) for l in range(L)]
        wk_sb = [load(f"wk{l}", wk_d[l], [128, 2 * E], BF16) for l in range(L)]
        wv_sb = [load(f"wv{l}", wv_d[l], [128, 2 * E], BF16) for l in range(L)]
        wo_sb = [load(f"wo{l}", wo_d[l], [96, 8 * E], BF16) for l in range(L)]
        w1_sb = [load(f"w1{l}", w1_d[l], [128, 2 * FF], BF16) for l in range(L)]
        w2_sb = [load(f"w2{l}", w2_d[l], [128, 8 * E], BF16) for l in range(L)]
        ln1s_sb = [load(f"ln1s{l}", ln1s_d[l], [128, 2]) for l in range(L)]
        ln1b_sb = [load(f"ln1b{l}", ln1b_d[l], [128, 2]) for l in range(L)]
        ln2s_sb = [load(f"ln2s{l}", ln2s_d[l], [128, 2]) for l in range(L)]
        ln2b_sb = [load(f"ln2b{l}", ln2b_d[l], [128, 2]) for l in range(L)]
        bqf_sb = [load(f"bqf{l}", bqf_d[l], [128, 2]) for l in range(L)]
        bkf_sb = [load(f"bkf{l}", bkf_d[l], [128, 2]) for l in range(L)]
        b1f_sb = [load(f"b1f{l}", b1f_d[l], [128, 8]) for l in range(L)]
        wob_sb = [load(f"wob{l}", wob_d[l], [1, E], BF16) for l in range(L)]
        wof_sb = [load(f"wof{l}", wof_d[l], [32, 8 * E], BF16) for l in range(L)]
        b2r_sb = [load(f"b2r{l}", b2r_d[l], [1, E], BF16) for l in range(L)]
        wc1_sb = load("wc1", wc1_d, [128, 2 * E], BF16)
        bc1_sb = load("bc1", bc1_d, [1, E], BF16)
        lncg_sb = load("lncg", lncg_d, [1, E])
        lncb_sb = load("lncb", lncb_d, [1, E])
        wc2_sb = load("wc2", wc2_d, [128, 2 * NCLS], BF16)
        bc2_sb = load("bc2", bc2_d, [1, NCLS], BF16)

        ident = cp.tile([128, 128], BF16, tag="ident")
        make_identity(nc, ident[:])
        ones_row = cp.tile([1, 128], BF16, tag="ones_row")
        nc.vector.memset(ones_row[:], 1.0)
        ones_col = cp.tile([128, 1], BF16, tag="ones_col")
        nc.vector.memset(ones_col[:], 1.0)

        # residual stream
        h = cp.tile([128, NQ * E], F32, tag="h")
        nc.vector.memset(h[:, 8 * E:], 0.0)

        # ---------------- embedding ----------------
        xT = ap_.tile([C, N], F32, tag="xT")
        nc.sync.dma_start(xT[:], x_d[:].rearrange("n c -> c n"))
        xTb = ap_.tile([C, N], BF16, tag="xTb")
        nc.vector.tensor_copy(xTb[:], xT[:])

        embg = ap_.tile([128, 8 * E], F32, tag="gt")   # shares slot with GT
        st6 = ap_.tile([128, 6 * NQ], F32, tag="st6")
        agg = ap_.tile([128, 2 * NQ], F32, tag="agg")
        rstd = ap_.tile([128, NQ], F32, tag="rstd")
        nmr = ap_.tile([128, NQ], F32, tag="nmr")
        tmpa = ap_.tile([128, NQ], F32, tag="tmpa")

        for j in range(8):
            ep = ps2.tile([128, E], F32, tag="misc")
            nc.tensor.matmul(ep[:], xTb[:, 128 * j:128 * (j + 1)], wemb_sb[:],
                             start=True, stop=False)
            nc.tensor.matmul(ep[:], ones_row[0:1, 0:128], bemb_sb[:],
                             start=False, stop=True)
            # LN stats for this tile
            nc.vector.bn_stats(st6[:, 6 * j:6 * j + 6], ep[:])
            nc.vector.bn_aggr(agg[:, 2 * j:2 * j + 2], st6[:, 6 * j:6 * j + 6])
            # stash raw emb (fp32) temporarily in embg
            nc.vector.tensor_copy(embg[:, E * j:E * (j + 1)], ep[:])
        agg3 = agg[:].rearrange("p (j t) -> p t j", t=2)
        nc.vector.tensor_scalar_add(tmpa[:, 0:8], agg3[:, 1:2, 0:8], EPS)
        nc.vector.reciprocal_approx_fast(rstd[:, 0:8], tmpa[:, 0:8])
        nc.scalar.activation(rstd[:, 0:8], rstd[:, 0:8], AF.Sqrt)
        nc.vector.tensor_tensor(out=nmr[:, 0:8], in0=agg3[:, 0:1, 0:8],
                                in1=rstd[:, 0:8], op=OP.mult)
        nc.vector.tensor_scalar_mul(nmr[:, 0:8], nmr[:, 0:8], -1.0)
        for j in range(8):
            sl = slice(E * j, E * (j + 1))
            # (x - m) * rstd, then * g + b (broadcast consts), then gelu
            nc.vector.tensor_scalar(out=embg[:, sl], in0=embg[:, sl],
                                    scalar1=agg3[:, 0:1, j:j + 1],
                                    scalar2=rstd[:, j:j + 1],
                                    op0=OP.subtract, op1=OP.mult)
            nc.vector.tensor_tensor(out=embg[:, sl], in0=embg[:, sl],
                                    in1=gemb_sb[:], op=OP.mult)
            nc.vector.tensor_tensor(out=embg[:, sl], in0=embg[:, sl],
                                    in1=beemb_sb[:], op=OP.add)
            nc.scalar.activation(embg[:, sl], embg[:, sl], AF.Gelu)
        # shift into h: h token 128j+p+1 <- emb token 128j+p
        for j in range(NQ):
            if j < 8:
                nc.sync.dma_start(h[1:128, E * j:E * j + E],
                                  embg[0:127, E * j:E * j + E])
            if j >= 1:
                nc.sync.dma_start(h[0:1, E * j:E * j + E],
                                  embg[127:128, E * (j - 1):E * j])
        nc.sync.dma_start(h[0:1, 0:E], cls_d[:])

        # ---------------- helpers ----------------
        def layer_norm(lns, lnb, ytA, ytB):
            """token-major LN of h -> feature-major bf16 [128,1025] x2."""
            y0 = ap_.tile([128, NQ * E], BF16, tag="y0")
            for j in range(NQ):
                nc.vector.bn_stats(st6[:, 6 * j:6 * j + 6],
                                   h[:, E * j:E * (j + 1)])
                nc.vector.bn_aggr(agg[:, 2 * j:2 * j + 2],
                                  st6[:, 6 * j:6 * j + 6])
            a3 = agg[:].rearrange("p (j t) -> p t j", t=2)
            nc.vector.tensor_scalar_add(tmpa[:], a3[:, 1:2, :], EPS)
            nc.vector.reciprocal_approx_fast(rstd[:], tmpa[:])
            nc.scalar.activation(rstd[:], rstd[:], AF.Sqrt)
            nc.vector.tensor_tensor(out=nmr[:], in0=a3[:, 0:1, :], in1=rstd[:],
                                    op=OP.mult)
            nc.vector.tensor_scalar_mul(nmr[:], nmr[:], -1.0)
            for j in range(NQ):
                nc.scalar.activation(y0[:, E * j:E * (j + 1)],
                                     h[:, E * j:E * (j + 1)], AF.Identity,
                                     bias=nmr[:, j:j + 1],
                                     scale=rstd[:, j:j + 1])
            # transpose + affine evac
            for t, yt in ((0, ytA), (1, ytB)):
                for jb in range(3):
                    js = list(range(4 * jb, min(4 * jb + 4, NQ)))
                    tp = ps2.tile([128, 512], BF16, tag="misc")
                    for i, j in enumerate(js):
                        nc.tensor.transpose(
                            tp[:, 128 * i:128 * (i + 1)],
                            y0[:, E * j + 128 * t:E * j + 128 * t + 128],
                            ident[:])
                    w = 128 * len(js) if jb < 2 else 1
                    nc.scalar.activation(yt[:, 512 * jb:512 * jb + w],
                                         tp[:, 0:w], AF.Identity,
                                         bias=lnb[:, t:t + 1],
                                         scale=lns[:, t:t + 1])

        def project_qk(w_sb, bias_fm, ys, qtA, qtB):
            """yT @ W -> feature-major [2][128,1025] bf16 with bias."""
            for m, qt in ((0, qtA), (1, qtB)):
                for c0, cw in ((0, 512), (512, 512), (1024, 1)):
                    pp = ps2.tile([128, 512], F32, tag="misc")
                    for t in range(2):
                        nc.tensor.matmul(
                            pp[:, 0:cw],
                            w_sb[:, E * t + 128 * m:E * t + 128 * m + 128],
                            ys[t][:, c0:c0 + cw],
                            start=(t == 0), stop=(t == 1))
                    nc.scalar.activation(qt[:, c0:c0 + cw], pp[:, 0:cw],
                                         AF.Identity,
                                         bias=bias_fm[:, m:m + 1], scale=1.0)

        # ---------------- transformer layers ----------------
        for l in range(L):
            # h += pos (gpsimd: keeps DVE free)
            nc.gpsimd.tensor_tensor(out=h[:], in0=h[:], in1=pos_sb[:],
                                    op=OP.add)
            # ---- attention ----
            ytA = ap_.tile([128, S], BF16, tag="ytA")
            ytB = ap_.tile([128, S], BF16, tag="ytB")
            layer_norm(ln1s_sb[l], ln1b_sb[l], ytA, ytB)

            qtA = ap_.tile([128, S], BF16, tag="qtA")
            qtB = ap_.tile([128, S], BF16, tag="qtB")
            ktA = ap_.tile([128, S], BF16, tag="ktA")
            ktB = ap_.tile([128, S], BF16, tag="ktB")
            project_qk(wq_sb[l], bqf_sb[l], (ytA, ytB), qtA, qtB)
            project_qk(wk_sb[l], bkf_sb[l], (ytA, ytB), ktA, ktB)

            # V token-major with 16.0 augmentation columns
            vsb = ap_.tile([128, NKT * 264], BF16, tag="vsb")
            v4 = vsb[:].rearrange("p (k s e) -> p k s e", k=NKT, s=8)
            nc.vector.memset(v4[:, :, :, 32:33], DIV)
            for kt in range(NKT):
                mw = 128 if kt < 8 else 1
                vp = ps2.tile([128, E], F32, tag="misc")
                for t in range(2):
                    nc.tensor.matmul(
                        vp[0:mw, :],
                        (ytA if t == 0 else ytB)[:, 128 * kt:128 * kt + mw],
                        wv_sb[l][:, E * t:E * (t + 1)],
                        start=(t == 0), stop=(t == 1))
                nc.vector.tensor_copy(
                    v4[0:mw, kt, :, 0:32],
                    vp[0:mw, :].rearrange("p (s d) -> p s d", s=8))

            # attention slots (serial over slots)
            otp = []
            for s in range(8):
                ot_s = ap_.tile([97, S], BF16, tag=f"otp{s}")
                otp.append(ot_s)

            def slot_rows(s):
                qt = qtA if s < 4 else qtB
                kt_t = ktA if s < 4 else ktB
                rp = 32 * (s % 4)
                return qt[rp:rp + 32, :], kt_t[rp:rp + 32, :], rp

            # straggler query (token 1024)
            stot = psM.tile([128, 96], F32, tag="stot")
            if KSTRAG:
                nc.vector.memset(stot[:, 0:96], 0.0)
                for kt in range(NKT):
                    mw = 128 if kt < 8 else 1
                    for s in range(8):
                        qrows, krows, rp = slot_rows(s)
                        nc.tensor.matmul(
                            stot[0:mw, 9 * s + kt:9 * s + kt + 1],
                            krows[:, 128 * kt:128 * kt + mw],
                            qrows[:, 1024:1025],
                            start=True, stop=True,
                            tile_position=(rp, 0))
                pts = ptp.tile([128, 72], BF16, tag="pts")
                nc.scalar.activation(pts[:], stot[:, 0:72], AF.Exp)
                for s in range(8):
                    ob = (0 if s % 2 == 0 else 64) if KPAIR else 0
                    for kt in range(NKT):
                        mw = 128 if kt < 8 else 1
                        nc.tensor.matmul(stot[ob:ob + 33, 80 + s:81 + s],
                                         vsb[0:mw, 264 * kt + 33 * s:
                                             264 * kt + 33 * s + 33],
                                         pts[0:mw, 9 * s + kt:9 * s + kt + 1],
                                         start=(kt == 0), stop=(kt == 8),
                                         tile_position=(0, ob))

            dnP = ap_.tile([128, S], F32, tag="dnP")
            if KPAIR:
                pairs = [(2 * p, 2 * p + 1) for p in range(4)]
            else:
                pairs = [(s, None) for s in range(8)]
            for sA, sB in pairs:
                pv = psM.tile([128, 1024], F32, tag="pv")
                group = []
                for s in ([sA] if sB is None else [sA, sB]):
                    qr, kr, rp = slot_rows(s)
                    ob = 0 if (sB is None or s == sA) else 64
                    group.append((s, qr, kr, rp, ob))
                for kt in range(NKT):
                    mw = 128 if kt < 8 else 1
                    for c0 in (0, 512):
                        # energies for both slots adjacent: different PE row
                        # groups -> run concurrently in the array
                        engs = []
                        for s, qr, kr, rp, ob in group:
                            eps_t = psE.tile([128, 512], F32, tag="eng")
                            nc.tensor.matmul(
                                eps_t[0:mw, :],
                                kr[:, 128 * kt:128 * kt + mw],
                                qr[:, c0:c0 + 512],
                                start=True, stop=True,
                                tile_position=(rp, 0))
                            engs.append(eps_t)
                        pts_c = []
                        for (s, qr, kr, rp, ob), eps_t in zip(group, engs):
                            ptt = ptp.tile([128, 512], BF16, tag="pt")
                            # slot-parity engine split: odd slots take DVE for
                            # kts in DVEK so both PV inputs arrive together
                            if s % 2 == 1 and kt in DVEK:
                                etmp = dnp.tile([128, 512], F32, tag="etmp")
                                nc.vector._custom_dve(
                                    EXPP, out=etmp[:], in0=eps_t[:],
                                    s0=EXPA3, s1=EXPA2, imm2=EXPA1)
                                nc.vector._custom_dve(SQ16, out=ptt[:],
                                                      in0=etmp[:])
                            else:
                                nc.scalar.activation(ptt[0:mw, :],
                                                     eps_t[0:mw, :], AF.Exp)
                            pts_c.append(ptt)
                        # PV for both slots adjacent: different col groups
                        for (s, qr, kr, rp, ob), ptt in zip(group, pts_c):
                            nc.tensor.matmul(
                                pv[ob:ob + 33, c0:c0 + 512],
                                vsb[0:mw, 264 * kt + 33 * s:
                                    264 * kt + 33 * s + 33],
                                ptt[0:mw, :],
                                start=(kt == 0), stop=(kt == 8),
                                tile_position=(0, ob))
                if not KSTRAG:
                    for s, qr, kr, rp, ob in group:
                        for kt in range(NKT):
                            mw = 128 if kt < 8 else 1
                            nc.tensor.matmul(
                                stot[0:mw, 9 * s + kt:9 * s + kt + 1],
                                kr[:, 128 * kt:128 * kt + mw],
                                qr[:, 1024:1025],
                                start=True, stop=True,
                                tile_position=(rp, 0))
                        ptsl = ptp.tile([128, NKT], BF16, tag="pts")
                        nc.scalar.activation(ptsl[:],
                                             stot[:, 9 * s:9 * s + NKT],
                                             AF.Exp)
                        for kt in range(NKT):
                            mw = 128 if kt < 8 else 1
                            nc.tensor.matmul(
                                stot[ob:ob + 33, 80 + s:81 + s],
                                vsb[0:mw, 264 * kt + 33 * s:
                                    264 * kt + 33 * s + 33],
                                ptsl[0:mw, kt:kt + 1],
                                start=(kt == 0), stop=(kt == 8),
                                tile_position=(0, ob))
                # normalize + evacuate: reciprocal straight off the PSUM
                # denominator rows (32 / 96), DMA each row to a partition-0
                # tile, gpsimd-broadcast, fused multiply-evac.
                for s, qr_, kr_, rp_, ob in group:
                    dr = ob + 32
                    nc.vector.tensor_copy(dnP[dr:dr + 1, 0:1024],
                                          pv[dr:dr + 1, :])
                    nc.vector.tensor_copy(dnP[dr:dr + 1, 1024:1025],
                                          stot[dr:dr + 1, 80 + s:81 + s])
                    dnQ = dnp.tile([1, S], F32, tag="dnQ")
                    nc.gpsimd.dma_start(dnQ[0:1, :], dnP[dr:dr + 1, :])
                    rcp = dnp.tile([1, S], F32, tag="rcp")
                    nc.vector.reciprocal_approx_fast(rcp[:], dnQ[:])
                    bc = bcp.tile([32, S], F32, tag="bc")
                    nc.gpsimd.partition_broadcast(bc[:], rcp[:])
                    nc.vector.tensor_tensor(out=otp[s][ob:ob + 32, 0:1024],
                                            in0=pv[ob:ob + 32, :],
                                            in1=bc[:, 0:1024], op=OP.mult)
                    nc.vector.tensor_tensor(out=otp[s][ob:ob + 32, 1024:1025],
                                            in0=stot[ob:ob + 32, 80 + s:81 + s],
                                            in1=bc[:, 1024:1025], op=OP.mult)

            # Wo projection + residual (+ bo + bv@Wo row). Even slots sit
            # at otp rows 0..31 (PE row group 0), odd at rows 64..95 (group
            # 2); cross-group accumulation into one PSUM region is not
            # supported, so use two accumulators and two residual adds.
            for qt_i in range(NQ):
                mw = 128 if qt_i < 8 else 1
                wp = ps2.tile([128, E], F32, tag="misc")
                if KPAIR:
                    wp2 = psE.tile([128, E], F32, tag="eng")
                    for s in range(0, 8, 2):
                        nc.tensor.matmul(
                            wp[0:mw, :],
                            otp[s][0:32, 128 * qt_i:128 * qt_i + mw],
                            wo_sb[l][0:32, E * s:E * (s + 1)],
                            start=(s == 0), stop=False,
                            tile_position=(0, 0))
                        nc.tensor.matmul(
                            wp2[0:mw, :],
                            otp[s + 1][64:96, 128 * qt_i:128 * qt_i + mw],
                            wo_sb[l][64:96, E * (s + 1):E * (s + 2)],
                            start=(s == 0), stop=(s == 6),
                            tile_position=(64, 0))
                else:
                    for s in range(8):
                        nc.tensor.matmul(
                            wp[0:mw, :],
                            otp[s][0:32, 128 * qt_i:128 * qt_i + mw],
                            wof_sb[l][0:32, E * s:E * (s + 1)],
                            start=(s == 0), stop=False,
                            tile_position=(0, 0))
                nc.tensor.matmul(wp[0:mw, :], ones_row[0:1, 0:mw],
                                 wob_sb[l][:], start=False, stop=True,
                                 tile_position=(0, 0))
                sl = slice(E * qt_i, E * (qt_i + 1))
                nc.vector.tensor_tensor(out=h[0:mw, sl], in0=h[0:mw, sl],
                                        in1=wp[0:mw, :], op=OP.add)
                if KPAIR:
                    nc.vector.tensor_tensor(out=h[0:mw, sl], in0=h[0:mw, sl],
                                            in1=wp2[0:mw, :], op=OP.add)

            # ---- MLP ----
            ytA = ap_.tile([128, S], BF16, tag="ytA")
            ytB = ap_.tile([128, S], BF16, tag="ytB")
            layer_norm(ln2s_sb[l], ln2b_sb[l], ytA, ytB)

            gt = ap_.tile([128, 8 * S], BF16, tag="gt")
            for f in range(8):
                for c0, cw in ((0, 512), (512, 512), (1024, 1)):
                    gp = ps2.tile([128, 512], F32, tag="misc")
                    for t in range(2):
                        nc.tensor.matmul(
                            gp[:, 0:cw],
                            w1_sb[l][:, FF * t + 128 * f:FF * t + 128 * f + 128],
                            (ytA if t == 0 else ytB)[:, c0:c0 + cw],
                            start=(t == 0), stop=(t == 1))
                    nc.scalar.activation(gt[:, S * f + c0:S * f + c0 + cw],
                                         gp[:, 0:cw], AF.Gelu,
                                         bias=b1f_sb[l][:, f:f + 1], scale=1.0)
            for qt_i in range(NQ):
                mw = 128 if qt_i < 8 else 1
                wp = ps2.tile([128, E], F32, tag="misc")
                for f in range(8):
                    nc.tensor.matmul(wp[0:mw, :],
                                     gt[:, S * f + 128 * qt_i:
                                        S * f + 128 * qt_i + mw],
                                     w2_sb[l][:, E * f:E * (f + 1)],
                                     start=(f == 0), stop=False)
                nc.tensor.matmul(wp[0:mw, :], ones_row[0:1, 0:mw],
                                 b2r_sb[l][:], start=False, stop=True)
                sl = slice(E * qt_i, E * (qt_i + 1))
                nc.vector.tensor_tensor(out=h[0:mw, sl], in0=h[0:mw, sl],
                                        in1=wp[0:mw, :], op=OP.add)

        # ---------------- classifier ----------------
        hbf = ap_.tile([128, NQ * E], BF16, tag="hbf")
        nc.vector.tensor_copy(hbf[:], h[:])
        mp = ps2.tile([1, E], F32, tag="misc")
        for j in range(8):
            nc.tensor.matmul(mp[:], ones_col[:], hbf[:, E * j:E * (j + 1)],
                             start=(j == 0), stop=False)
        nc.tensor.matmul(mp[:], ones_col[0:1, :], hbf[0:1, 8 * E:9 * E],
                         start=False, stop=True)
        pbf = dnp.tile([1, E], BF16, tag="pbf")
        nc.scalar.activation(pbf[:], mp[:], AF.Identity, bias=0.0,
                             scale=1.0 / S)
        # p @ Wc1 + bc1
        ptp_ps = ps2.tile([128, 512], BF16, tag="misc")
        pT = dnp.tile([128, 2], BF16, tag="pT")
        for t in range(2):
            nc.tensor.transpose(ptp_ps[:, 128 * t:128 * t + 1],
                                pbf[0:1, 128 * t:128 * (t + 1)],
                                ident[0:1, 0:1])
            nc.vector.tensor_copy(pT[:, t:t + 1],
                                  ptp_ps[:, 128 * t:128 * t + 1])
        c1p = ps2.tile([1, E], F32, tag="misc")
        for t in range(2):
            nc.tensor.matmul(c1p[:], pT[:, t:t + 1],
                             wc1_sb[:, E * t:E * (t + 1)],
                             start=(t == 0), stop=False)
        nc.tensor.matmul(c1p[:], ones_row[0:1, 0:1], bc1_sb[:],
                         start=False, stop=True)
        # LN over the [1, E] row
        p2 = dnp.tile([1, E], F32, tag="p2")
        nc.vector.tensor_copy(p2[:], c1p[:])
        st1 = dnp.tile([1, 6], F32, tag="st1")
        ag1 = dnp.tile([1, 2], F32, tag="ag1")
        nc.vector.bn_stats(st1[:], p2[:])
        nc.vector.bn_aggr(ag1[:], st1[:])
        r1 = dnp.tile([1, 2], F32, tag="r1")
        nc.vector.tensor_scalar_add(r1[:, 0:1], ag1[:, 1:2], EPS)
        nc.vector.reciprocal_approx_fast(r1[:, 0:1], r1[:, 0:1])
        nc.scalar.activation(r1[:, 0:1], r1[:, 0:1], AF.Sqrt)
        nc.vector.tensor_scalar(out=p2[:], in0=p2[:], scalar1=ag1[:, 0:1],
                                scalar2=r1[:, 0:1], op0=OP.subtract,
                                op1=OP.mult)
        nc.vector.tensor_tensor(out=p2[:], in0=p2[:], in1=lncg_sb[:],
                                op=OP.mult)
        nc.vector.tensor_tensor(out=p2[:], in0=p2[:], in1=lncb_sb[:],
                                op=OP.add)
        p2b = dnp.tile([1, E], BF16, tag="p2b")
        nc.vector.tensor_copy(p2b[:], p2[:])
        p2T = dnp.tile([128, 2], BF16, tag="p2T")
        for t in range(2):
            tp2 = ps2.tile([128, 512], BF16, tag="misc")
            nc.tensor.transpose(tp2[:, 0:1], p2b[0:1, 128 * t:128 * (t + 1)],
                                ident[0:1, 0:1])
            nc.vector.tensor_copy(p2T[:, t:t + 1], tp2[:, 0:1])
        op_ = ps2.tile([1, NCLS], F32, tag="misc")
        for t in range(2):
            nc.tensor.matmul(op_[:], p2T[:, t:t + 1],
                             wc2_sb[:, NCLS * t:NCLS * (t + 1)],
                             start=(t == 0), stop=False)
        nc.tensor.matmul(op_[:], ones_row[0:1, 0:1], bc2_sb[:],
                         start=False, stop=True)
        osb = dnp.tile([1, NCLS], F32, tag="osb")
        nc.vector.tensor_copy(osb[:], op_[:])
        nc.sync.dma_start(out_d[:], osb[:])

        for _p in (ps2, psE, psM, dnp, bcp, ptp, ap_, cp):
            _p.release()

    nc.compile()
    return nc


def _prep_shared(inputs):
    """Host-side weight preparation (shared across cores)."""
    bf16 = ml_dtypes.bfloat16
    f32 = np.float32
    g = {k: np.asarray(v, dtype=f32) for k, v in inputs.items()}
    d = {}

    pos_tm = np.zeros((128, NQ * E), f32)
    for j in range(NQ):
        n = 128 if j < 8 else 1
        pos_tm[0:n, E * j:E * (j + 1)] = g["pos"][128 * j:128 * j + n]
    d["pos_tm"] = pos_tm
    d["cls_row"] = g["cls_token"].reshape(1, E)
    d["wemb"] = g["W_emb"].astype(bf16)
    d["bemb_row"] = g["b_emb"].reshape(1, E).astype(bf16)
    d["gemb_bc"] = np.broadcast_to(g["g_emb"], (128, E)).copy()
    d["beemb_bc"] = np.broadcast_to(g["be_emb"], (128, E)).copy()

    perm = np.concatenate([np.arange(32) + 32 * SIG[s] for s in range(8)])

    def pack_k(w):  # [256, X] -> [128, 2X]
        return np.concatenate([w[0:128], w[128:256]], axis=1)

    for l in range(L):
        d[f"wq{l}"] = pack_k(g["Wq"][l][:, perm]).astype(bf16)
        d[f"wk{l}"] = pack_k(g["Wk"][l][:, perm]).astype(bf16)
        d[f"wv{l}"] = pack_k(g["Wv"][l][:, perm]).astype(bf16)
        # wo: [96, 8*E]; slot s cols E*s hold Wo rows of head SIG[s], at
        # partition rows 0..31 for even s and 64..95 for odd s.
        wo = np.zeros((96, 8 * E), f32)
        for s in range(8):
            r0 = 0 if s % 2 == 0 else 64
            wo[r0:r0 + 32, E * s:E * (s + 1)] = \
                g["Wo"][l][32 * SIG[s]:32 * SIG[s] + 32]
        d[f"wo{l}"] = wo.astype(bf16)
        wof = np.zeros((32, 8 * E), f32)
        for s in range(8):
            wof[:, E * s:E * (s + 1)] = \
                g["Wo"][l][32 * SIG[s]:32 * SIG[s] + 32]
        d[f"wof{l}"] = wof.astype(bf16)
        d[f"w1{l}"] = pack_k(g["W1"][l]).astype(bf16)
        w2 = np.concatenate([g["W2"][l][128 * t:128 * (t + 1)]
                             for t in range(8)], axis=1)
        d[f"w2{l}"] = w2.astype(bf16)
        d[f"ln1s{l}"] = pack_k(g["ln1_g"][l].reshape(E, 1))
        d[f"ln1b{l}"] = pack_k(g["ln1_b"][l].reshape(E, 1))
        d[f"ln2s{l}"] = pack_k(g["ln2_g"][l].reshape(E, 1))
        d[f"ln2b{l}"] = pack_k(g["ln2_b"][l].reshape(E, 1))
        d[f"bqf{l}"] = pack_k(g["bq"][l][perm].reshape(E, 1))
        d[f"bkf{l}"] = pack_k(g["bk"][l][perm].reshape(E, 1))
        d[f"b1f{l}"] = np.stack([g["b1"][l][128 * t:128 * (t + 1)]
                                 for t in range(8)], axis=1)
        d[f"wob{l}"] = (g["bo"][l] + g["bv"][l] @ g["Wo"][l]).reshape(1, E).astype(bf16)
        d[f"b2r{l}"] = g["b2"][l].reshape(1, E).astype(bf16)
    d["wc1"] = pack_k(g["Wc1"]).astype(bf16)
    d["bc1r"] = g["bc1"].reshape(1, E).astype(bf16)
    d["lncg"] = g["lnc_g"].reshape(1, E)
    d["lncb"] = g["lnc_b"].reshape(1, E)
    d["wc2"] = pack_k(g["Wc2"]).astype(bf16)
    d["bc2r"] = g["bc2"].reshape(1, NCLS).astype(bf16)
    return d


def kernel(**inputs):
    sys.path.insert(0, TRN_REPO)
    from concourse.bass_utils import run_bass_kernel_spmd

    if "nc" not in _CACHE:
        _CACHE["nc"] = _build()
    nc = _CACHE["nc"]

    shared = _prep_shared(inputs)
    x = np.asarray(inputs["x"], dtype=np.float32)
    in_maps = [dict(shared, x=np.ascontiguousarray(x[c])) for c in range(B)]
    res = run_bass_kernel_spmd(nc, in_maps, list(range(B)))
    out = np.stack([np.asarray(res.results[c]["out"]).reshape(NCLS)
                    for c in range(B)])
    return out.astype(np.float32)



# revision 26
# speedup vs baseline: 1.3983x; 1.3983x over previous
"""ECGformer forward pass on 8 TRN2 NeuronCores, data-parallel over batch.

Layout strategy per core (1 batch element):
- Residual stream h: token-major fp32, [128, 9*256]; tokens 0..1024 at
  (j=tok//128, p=tok%128); tile-8 rows 1..127 are kept EXACTLY zero
  (residual evacs for tile 8 only write row 0), so key-padding to 1152
  is safe and bounded.
- Keys padded to 1152 (9 uniform 128-token k-tiles); V carries a 16.0
  augmentation column per slot whose pad rows are zeroed once, so pad
  keys contribute nothing to numerator or denominator.
- Attention in [k, q] feature-major; q processed in passes (1024,1),
  (0,512), (512,512); energies for 3 consecutive k-tiles land in one
  [128, 3*512] PSUM tile so softmax exp runs as one wide op (amortizes
  per-op overhead); exp split between ScalarE (table exp) and VectorE
  (single fused custom op: (1+c1 x+c2 x^2)^16, or 2-op cubic chain).
- PV accumulated per (pair of slots) into a double-buffered [128,512]
  PSUM bank; normalization: reciprocal of the augmentation row +
  gpsimd partition-broadcast + one fused multiply into bf16 otpP.
- Weights land via 7 large DMAs (per-layer bf16 blobs + fp32 consts)
  instead of ~118 small ones, so compute starts almost immediately.
"""

import os
import sys

import numpy as np

try:
    import ml_dtypes
except ImportError:  # pragma: no cover
    ml_dtypes = None

TRN_REPO = "/opt/trn_rl_repo"

B, N, C = 8, 1024, 12
E, H, HD, L, FF, NCLS = 256, 8, 32, 4, 1024, 5
S = N + 1          # 1025 real tokens
NQ = 9             # token tiles (tile 8 = token 1024 + 127 pad rows)
SP = NQ * 128      # 1152 padded tokens (keys)
EPS = 1e-5
SIG = [0, 4, 1, 5, 2, 6, 3, 7]   # slot s holds head SIG[s]
DIV = 16.0         # sqrt(E); folded into the V augmentation column

_CACHE = {}

# exp approximations (DVE). cubic chain: ((a3 x + a2) x + a1) x + 1)^16
EXPA1, EXPA2, EXPA3 = 0.0625147850914497, 0.0019667051147439383, 4.0039417979884645e-05
# fused quad: (1 + q1 x + q2 x^2)^16  (minimax fit on [-5.8, 5.8])
EXPQ1, EXPQ2 = 0.06346399, 0.00194877
EXPMODE = os.environ.get("EXPMODE", "quad")   # "quad" | "cubic2"
# which exp tiles go to DVE: (slot parity odd) and group in DVEG
DVEG = {int(v) for v in os.environ.get("DVEG", "0,1,2").split(",") if v != ""}

QC_PROJ = [(0, 512), (512, 512), (1024, 128)]
QC_ATT = [(1024, 1), (0, 512), (512, 512)]
KG = [(0, 3), (3, 3), (6, 3)]   # k-tile groups of 3


def _register_exp_ops():
    import re
    from concourse import dve_ops as D
    from concourse.dve_spec import Spec, Src0, C0, C1, C2, One, sq
    if any(op.name == "ANT_EXPP" for op in D.OPS):
        return
    expp = D.DveOp(
        name="ANT_EXPP",
        spec=Spec(
            body=((C0 * Src0 + C1) * Src0 + C2) * Src0 + One,
            reference=lambda in0, in1, s0, s1, imm2:
                ((s0 * in0 + s1) * in0 + imm2) * in0 + 1.0,
        ),
        subdim=False, uops_sha={})
    sq16 = D.DveOp(
        name="ANT_SQ16",
        spec=Spec(body=sq(sq(sq(sq(Src0)))),
                  reference=lambda in0, in1, s0, s1, imm2:
                      (((in0.astype(np.float32) ** 2) ** 2) ** 2) ** 2),
        subdim=False, uops_sha={})
    expq = D.DveOp(
        name="ANT_EXPQ",
        spec=Spec(
            body=sq(sq(sq(sq((C0 * Src0 + C1) * Src0 + One)))),
            reference=lambda in0, in1, s0, s1, imm2:
                (((((s0 * in0 + s1) * in0 + 1.0) ** 2) ** 2) ** 2) ** 2,
        ),
        subdim=False, uops_sha={})
    for op in (expp, sq16, expq):
        D.OPS.append(op)
        D.CUSTOM_DVE_SPECS[op.name] = op.spec
        D._SUB_OPCODE_FOR_NAME[op.name] = D._CUSTOM_DVE_ROW_BASE + len(D.OPS) - 1
        assert D._SUB_OPCODE_FOR_NAME[op.name] < 0x20
        for ver in ("v3", "v4"):
            try:
                op.compile(ver)
            except ValueError as e:
                m = re.search(r"\(" + ver + r": ([0-9a-f]+)", str(e))
                op.uops_sha[ver] = m.group(1)
        op.compile("v3")


# ---- layer bf16 blob offsets (8192 cols) ----
OW_Q, OW_K, OW_V = 0, 512, 1024
OW_O = 1536          # [96, 2048]
OW_1 = 3584          # [128, 2048]
OW_2 = 5632          # [128, 2048]
OW_OB = 7680         # [1, 256]
OW_B2 = 7936         # [1, 256]
WBL_COLS = 8192
# ---- embw bf16 blob (1312 cols) ----
OE_WEMB, OE_BEMB, OE_WC1, OE_BC1, OE_WC2, OE_BC2 = 0, 256, 512, 1024, 1280, 1296
EMBW_COLS = 1312
# ---- cb fp32 blob (3664 cols) ----
OC_POS, OC_GEMB, OC_BEEMB, OC_CLS, OC_LNCG, OC_LNCB, OC_LAY = \
    0, 2304, 2560, 2816, 3072, 3328, 3584
CB_COLS = 3664


def _build():
    sys.path.insert(0, TRN_REPO)
    import concourse.tile as tile
    from concourse import mybir, bacc
    from concourse.masks import make_identity
    from concourse import dve_ops as D
    _register_exp_ops()
    EXPP = next(op for op in D.OPS if op.name == "ANT_EXPP")
    SQ16 = next(op for op in D.OPS if op.name == "ANT_SQ16")
    EXPQ = next(op for op in D.OPS if op.name == "ANT_EXPQ")

    F32 = mybir.dt.float32
    BF16 = mybir.dt.bfloat16
    AF = mybir.ActivationFunctionType
    OP = mybir.AluOpType

    nc = bacc.Bacc("TRN2", target_bir_lowering=False, debug=False, num_devices=8)

    x_d = nc.dram_tensor("x", [N, C], F32, kind="ExternalInput").ap()
    embw_d = nc.dram_tensor("embw", [128, EMBW_COLS], BF16, kind="ExternalInput").ap()
    cb_d = nc.dram_tensor("cb", [128, CB_COLS], F32, kind="ExternalInput").ap()
    wbl_d = [nc.dram_tensor(f"wbl{l}", [128, WBL_COLS], BF16, kind="ExternalInput").ap()
             for l in range(L)]
    out_d = nc.dram_tensor("out", [1, NCLS], F32, kind="ExternalOutput").ap()
    DBG = int(os.environ.get("DBG", "0"))
    if DBG:
        dbg_d = nc.dram_tensor("dbg", [128, 4 * NQ * E], F32,
                               kind="ExternalOutput").ap()

    with tile.TileContext(nc) as tc:
        cp = tc.alloc_tile_pool(name="consts", bufs=1)
        ap_ = tc.alloc_tile_pool(name="acts", bufs=1)
        ptp = tc.alloc_tile_pool(name="ptp", bufs=4)
        bcp = tc.alloc_tile_pool(name="bcp", bufs=2)
        dnp = tc.alloc_tile_pool(name="dnp", bufs=2)
        big = tc.alloc_tile_pool(name="big", bufs=2, space="PSUM")   # [128,1536]
        pvp = tc.alloc_tile_pool(name="pvp", bufs=2, space="PSUM")   # [128,512]

        # ---------------- weights into SBUF (few big DMAs) ----------------
        xT = ap_.tile([C, N], F32, tag="xT")
        nc.sync.dma_start(xT[:], x_d[:].rearrange("n c -> c n"))
        embw = cp.tile([128, EMBW_COLS], BF16, tag="embw")
        nc.sync.dma_start(embw[:], embw_d[:])
        cb = cp.tile([128, CB_COLS], F32, tag="cb")
        nc.sync.dma_start(cb[:], cb_d[:])
        wbl = []
        for l in range(L):
            t = cp.tile([128, WBL_COLS], BF16, tag=f"wbl{l}")
            eng = nc.sync if l % 2 == 0 else nc.gpsimd
            eng.dma_start(t[:], wbl_d[l][:])
            wbl.append(t)

        wemb_sb = embw[0:C, OE_WEMB:OE_WEMB + E]
        bemb_sb = embw[0:1, OE_BEMB:OE_BEMB + E]
        wc1_sb = embw[:, OE_WC1:OE_WC1 + 2 * E]
        bc1_sb = embw[0:1, OE_BC1:OE_BC1 + E]
        wc2_sb = embw[:, OE_WC2:OE_WC2 + 2 * NCLS]
        bc2_sb = embw[0:1, OE_BC2:OE_BC2 + NCLS]
        pos_sb = cb[:, OC_POS:OC_POS + NQ * E]
        gemb_sb = cb[:, OC_GEMB:OC_GEMB + E]
        beemb_sb = cb[:, OC_BEEMB:OC_BEEMB + E]
        cls_sb = cb[0:1, OC_CLS:OC_CLS + E]
        lncg_sb = cb[0:1, OC_LNCG:OC_LNCG + E]
        lncb_sb = cb[0:1, OC_LNCB:OC_LNCB + E]

        def lay32(l, off, w):
            return cb[:, OC_LAY + 20 * l + off:OC_LAY + 20 * l + off + w]
        ln1s_sb = [lay32(l, 0, 2) for l in range(L)]
        ln1b_sb = [lay32(l, 2, 2) for l in range(L)]
        ln2s_sb = [lay32(l, 4, 2) for l in range(L)]
        ln2b_sb = [lay32(l, 6, 2) for l in range(L)]
        bqf_sb = [lay32(l, 8, 2) for l in range(L)]
        bkf_sb = [lay32(l, 10, 2) for l in range(L)]
        b1f_sb = [lay32(l, 12, 8) for l in range(L)]
        wq_sb = [wbl[l][:, OW_Q:OW_Q + 2 * E] for l in range(L)]
        wk_sb = [wbl[l][:, OW_K:OW_K + 2 * E] for l in range(L)]
        wv_sb = [wbl[l][:, OW_V:OW_V + 2 * E] for l in range(L)]
        wo_sb = [wbl[l][0:96, OW_O:OW_O + 8 * E] for l in range(L)]
        w1_sb = [wbl[l][:, OW_1:OW_1 + 2 * FF] for l in range(L)]
        w2_sb = [wbl[l][:, OW_2:OW_2 + 8 * E] for l in range(L)]
        wob_sb = [wbl[l][0:1, OW_OB:OW_OB + E] for l in range(L)]
        b2r_sb = [wbl[l][0:1, OW_B2:OW_B2 + E] for l in range(L)]

        ident = cp.tile([128, 128], BF16, tag="ident")
        make_identity(nc, ident[:])
        ones_row = cp.tile([1, 128], BF16, tag="ones_row")
        nc.vector.memset(ones_row[:], 1.0)
        ones_col = cp.tile([128, 1], BF16, tag="ones_col")
        nc.vector.memset(ones_col[:], 1.0)

        # residual stream; tile-8 rows 1..127 stay exactly 0 forever
        h = cp.tile([128, NQ * E], F32, tag="h")
        nc.vector.memset(h[:, 8 * E:], 0.0)

        # ---------------- embedding ----------------
        xTb = ap_.tile([C, N], BF16, tag="xTb")
        nc.vector.tensor_copy(xTb[:], xT[:])

        embg = ap_.tile([128, 8 * E], F32, tag="gt")   # shares slot with gt
        st6 = ap_.tile([128, 6 * NQ], F32, tag="st6")
        agg = ap_.tile([128, 2 * NQ], F32, tag="agg")
        rstd = ap_.tile([128, NQ], F32, tag="rstd")
        tmpa = ap_.tile([128, NQ], F32, tag="tmpa")

        for j in range(8):
            ep = big.tile([128, 1536], F32, tag="big")
            nc.tensor.matmul(ep[:, 0:E], xTb[:, 128 * j:128 * (j + 1)], wemb_sb,
                             start=True, stop=False)
            nc.tensor.matmul(ep[:, 0:E], ones_row[0:1, 0:128], bemb_sb,
                             start=False, stop=True)
            nc.vector.bn_stats(st6[:, 6 * j:6 * j + 6], ep[:, 0:E])
            nc.vector.bn_aggr(agg[:, 2 * j:2 * j + 2], st6[:, 6 * j:6 * j + 6])
            nc.vector.tensor_copy(embg[:, E * j:E * (j + 1)], ep[:, 0:E])
        agg3 = agg[:].rearrange("p (j t) -> p t j", t=2)
        nc.vector.tensor_scalar_add(tmpa[:, 0:8], agg3[:, 1:2, 0:8], EPS)
        nc.vector.reciprocal_approx_fast(rstd[:, 0:8], tmpa[:, 0:8])
        nc.scalar.activation(rstd[:, 0:8], rstd[:, 0:8], AF.Sqrt)
        for j in range(8):
            sl = slice(E * j, E * (j + 1))
            nc.vector.tensor_scalar(out=embg[:, sl], in0=embg[:, sl],
                                    scalar1=agg3[:, 0:1, j:j + 1],
                                    scalar2=rstd[:, j:j + 1],
                                    op0=OP.subtract, op1=OP.mult)
            nc.vector.tensor_tensor(out=embg[:, sl], in0=embg[:, sl],
                                    in1=gemb_sb, op=OP.mult)
            nc.vector.tensor_tensor(out=embg[:, sl], in0=embg[:, sl],
                                    in1=beemb_sb, op=OP.add)
            nc.scalar.activation(embg[:, sl], embg[:, sl], AF.Gelu)
        # shift into h: h token 128j+p+1 <- emb token 128j+p
        for j in range(NQ):
            if j < 8:
                nc.sync.dma_start(h[1:128, E * j:E * j + E],
                                  embg[0:127, E * j:E * j + E])
            if j >= 1:
                nc.sync.dma_start(h[0:1, E * j:E * j + E],
                                  embg[127:128, E * (j - 1):E * j])
        nc.sync.dma_start(h[0:1, 0:E], cls_sb)

        # ---------------- helpers ----------------
        y0 = ap_.tile([128, NQ * E], BF16, tag="y0")

        def layer_norm(lns, lnb, ytA, ytB):
            """token-major LN of h -> feature-major bf16 [128,1152] x2."""
            for j in range(NQ):
                nc.vector.bn_stats(st6[:, 6 * j:6 * j + 6],
                                   h[:, E * j:E * (j + 1)])
                nc.vector.bn_aggr(agg[:, 2 * j:2 * j + 2],
                                  st6[:, 6 * j:6 * j + 6])
            a3 = agg[:].rearrange("p (j t) -> p t j", t=2)
            nc.vector.tensor_scalar_add(tmpa[:], a3[:, 1:2, :], EPS)
            nc.vector.reciprocal_approx_fast(rstd[:], tmpa[:])
            nc.scalar.activation(rstd[:], rstd[:], AF.Sqrt)
            for j in range(NQ):
                nc.vector.tensor_scalar(out=y0[:, E * j:E * (j + 1)],
                                        in0=h[:, E * j:E * (j + 1)],
                                        scalar1=a3[:, 0:1, j:j + 1],
                                        scalar2=rstd[:, j:j + 1],
                                        op0=OP.subtract, op1=OP.mult)
            # transpose + affine evac (feature-major, per-partition on ACT)
            for t, yt in ((0, ytA), (1, ytB)):
                for jb in range(3):
                    js = list(range(4 * jb, min(4 * jb + 4, NQ)))
                    tp = big.tile([128, 1536], BF16, tag="big")
                    for i, j in enumerate(js):
                        nc.tensor.transpose(
                            tp[:, 128 * i:128 * (i + 1)],
                            y0[:, E * j + 128 * t:E * j + 128 * t + 128],
                            ident[:])
                    w = 128 * len(js)
                    nc.scalar.activation(yt[:, 512 * jb:512 * jb + w],
                                         tp[:, 0:w], AF.Identity,
                                         bias=lnb[:, t:t + 1],
                                         scale=lns[:, t:t + 1])

        def project_qk(w_sb, bias_fm, ys, qtA, qtB):
            """yT @ W -> feature-major [2][128,1152] bf16 with bias."""
            for m, qt in ((0, qtA), (1, qtB)):
                for c0, cw in QC_PROJ:
                    pp = big.tile([128, 1536], F32, tag="big")
                    for t in range(2):
                        nc.tensor.matmul(
                            pp[:, 0:cw],
                            w_sb[:, E * t + 128 * m:E * t + 128 * m + 128],
                            ys[t][:, c0:c0 + cw],
                            start=(t == 0), stop=(t == 1))
                    nc.scalar.activation(qt[:, c0:c0 + cw], pp[:, 0:cw],
                                         AF.Identity,
                                         bias=bias_fm[:, m:m + 1], scale=1.0)

        # persistent attention tiles
        ytA = ap_.tile([128, SP], BF16, tag="ytA")
        ytB = ap_.tile([128, SP], BF16, tag="ytB")
        qtA = ap_.tile([128, SP], BF16, tag="qtA")
        qtB = ap_.tile([128, SP], BF16, tag="qtB")
        ktA = ap_.tile([128, SP], BF16, tag="ktA")
        ktB = ap_.tile([128, SP], BF16, tag="ktB")
        vsb = ap_.tile([128, NQ * 264 + 64], BF16, tag="vsb")
        otp = []
        for p in range(4):
            otp_p = ap_.tile([128, SP], BF16, tag=f"otp{p}")
            # cols beyond the 1025 real queries are read (discarded) by the
            # Wo matmul for token-tile 8 — keep them initialized
            nc.vector.memset(otp_p[:, S - 1:SP], 0.0)
            otp.append(otp_p)
        gt = ap_.tile([128, 8 * SP], BF16, tag="gt")

        v4 = vsb[:, 0:NQ * 264].rearrange("p (k s e) -> p k s e", k=NQ, s=8)
        # one-time: zero the k-tile-8 block + overread pad, then aug col =
        # 16.0 (k-tile 8: row 0 only) — V copies never touch the pad rows.
        nc.vector.memset(vsb[:, 8 * 264:9 * 264 + 64], 0.0)
        nc.vector.memset(v4[:, 0:8, :, 32:33], DIV)
        nc.vector.memset(v4[0:1, 8, :, 32:33], DIV)

        def slot_rows(s):
            qt = qtA if s < 4 else qtB
            kt_t = ktA if s < 4 else ktB
            rp = 32 * (s % 4)
            return qt[rp:rp + 32, :], kt_t[rp:rp + 32, :], rp

        # ---------------- transformer layers ----------------
        for l in range(L):
            # h += pos, per token tile (gpsimd; keeps DVE free)
            for j in range(NQ):
                nc.gpsimd.tensor_tensor(out=h[:, E * j:E * (j + 1)],
                                        in0=h[:, E * j:E * (j + 1)],
                                        in1=pos_sb[:, E * j:E * (j + 1)],
                                        op=OP.add)
            # ---- attention ----
            layer_norm(ln1s_sb[l], ln1b_sb[l], ytA, ytB)
            project_qk(wq_sb[l], bqf_sb[l], (ytA, ytB), qtA, qtB)
            project_qk(wk_sb[l], bkf_sb[l], (ytA, ytB), ktA, ktB)

            # V token-major into slot layout (tile 8: row 0 only)
            for kt in range(NQ):
                mw = 128 if kt < 8 else 1
                vp = big.tile([128, 1536], F32, tag="big")
                for t in range(2):
                    nc.tensor.matmul(
                        vp[0:mw, 0:E],
                        (ytA if t == 0 else ytB)[:, 128 * kt:128 * kt + mw],
                        wv_sb[l][:, E * t:E * (t + 1)],
                        start=(t == 0), stop=(t == 1))
                nc.vector.tensor_copy(
                    v4[0:mw, kt, :, 0:32],
                    vp[0:mw, 0:E].rearrange("p (s d) -> p s d", s=8))

            for c0, cw in QC_ATT:
                for pair in range(4):
                    sA, sB = 2 * pair, 2 * pair + 1
                    pv = pvp.tile([128, 512], F32, tag="pv")
                    group = []
                    for s in (sA, sB):
                        qr, kr, rp = slot_rows(s)
                        ob = 0 if s == sA else 64
                        group.append((s, qr, kr, rp, ob))
                    for g, (k0, kn) in enumerate(KG):
                        eng = []
                        for s, qr, kr, rp, ob in group:
                            eps_t = big.tile([128, 1536], F32, tag="big")
                            eng.append(eps_t)
                        for ki in range(kn):
                            kt = k0 + ki
                            for (s, qr, kr, rp, ob), eps_t in zip(group, eng):
                                nc.tensor.matmul(
                                    eps_t[:, 512 * ki:512 * ki + cw],
                                    kr[:, 128 * kt:128 * kt + 128],
                                    qr[:, c0:c0 + cw],
                                    start=True, stop=True,
                                    tile_position=(rp, 0))
                        pts_c = []
                        for (s, qr, kr, rp, ob), eps_t in zip(group, eng):
                            ptt = ptp.tile([128, 1536], BF16, tag="pt")
                            e3 = eps_t[:].rearrange("p (k w) -> p k w", k=3)
                            p3 = ptt[:].rearrange("p (k w) -> p k w", k=3)
                            if s % 2 == 1 and g in DVEG:
                                if EXPMODE == "quad":
                                    nc.vector._custom_dve(
                                        EXPQ, out=p3[:, :, 0:cw],
                                        in0=e3[:, :, 0:cw],
                                        s0=EXPQ2, s1=EXPQ1)
                                else:
                                    etmp = dnp.tile([128, 1536], F32,
                                                    tag="etmp")
                                    t3 = etmp[:].rearrange(
                                        "p (k w) -> p k w", k=3)
                                    nc.vector._custom_dve(
                                        EXPP, out=t3[:, :, 0:cw],
                                        in0=e3[:, :, 0:cw],
                                        s0=EXPA3, s1=EXPA2, imm2=EXPA1)
                                    nc.vector._custom_dve(
                                        SQ16, out=p3[:, :, 0:cw],
                                        in0=t3[:, :, 0:cw])
                            else:
                                nc.scalar.activation(p3[:, :, 0:cw],
                                                     e3[:, :, 0:cw], AF.Exp)
                            pts_c.append(ptt)
                        for ki in range(kn):
                            kt = k0 + ki
                            for (s, qr, kr, rp, ob), ptt in zip(group, pts_c):
                                nc.tensor.matmul(
                                    pv[ob:ob + 64, 0:cw],
                                    vsb[:, 264 * kt + 33 * s:
                                        264 * kt + 33 * s + 64],
                                    ptt[:, 512 * ki:512 * ki + cw],
                                    start=(kt == 0), stop=(kt == 8),
                                    tile_position=(0, ob),
                                    skip_group_check=True)
                    # normalize: pv -> SBUF copy, recip (full width — DVE
                    # cost is free-dim only), dma den rows, broadcast, mult
                    den = dnp.tile([128, 1536], F32, tag="etmp")
                    nc.vector.tensor_copy(den[:, 0:cw], pv[:, 0:cw])
                    nc.vector.reciprocal_approx_fast(den[:, 0:cw],
                                                     den[:, 0:cw])
                    rcpGA = dnp.tile([1, 512], F32, tag="rcpGA")
                    rcpGB = dnp.tile([1, 512], F32, tag="rcpGB")
                    nc.gpsimd.dma_start(rcpGA[0:1, 0:cw], den[32:33, 0:cw])
                    nc.gpsimd.dma_start(rcpGB[0:1, 0:cw], den[96:97, 0:cw])
                    bcA = bcp.tile([32, 512], F32, tag="bcA")
                    bcB = bcp.tile([32, 512], F32, tag="bcB")
                    nc.gpsimd.partition_broadcast(bcA[0:32, 0:cw],
                                                  rcpGA[0:1, 0:cw])
                    nc.gpsimd.partition_broadcast(bcB[0:32, 0:cw],
                                                  rcpGB[0:1, 0:cw])
                    nc.vector.tensor_tensor(out=otp[pair][0:32, c0:c0 + cw],
                                            in0=pv[0:32, 0:cw],
                                            in1=bcA[0:32, 0:cw], op=OP.mult)
                    nc.vector.tensor_tensor(out=otp[pair][64:96, c0:c0 + cw],
                                            in0=pv[64:96, 0:cw],
                                            in1=bcB[0:32, 0:cw], op=OP.mult)

            if DBG == 2 and l == 0:
                for di, t in enumerate((ytA, ytB, qtA, ktA, otp[0], otp[1],
                                        otp[2], otp[3])):
                    dcol = SP * di
                    dst = dbg_d[:, dcol:dcol + SP]
                    tmpd = ap_.tile([128, SP], F32, tag="dbgtmp")
                    nc.vector.tensor_copy(tmpd[:], t[:])
                    nc.sync.dma_start(dst, tmpd[:])

            # Wo projection + residual (+ bo + bv@Wo row). Even slots at
            # otp rows 0..31 (PE row group 0), odd at 64..95 (group 2).
            for qt_i in range(NQ):
                mw = 128 if qt_i < 8 else 1
                wp = big.tile([128, 1536], F32, tag="big")
                wp2 = pvp.tile([128, 512], F32, tag="pv")
                for p in range(4):
                    nc.tensor.matmul(
                        wp[:, 0:E],
                        otp[p][0:32, 128 * qt_i:128 * (qt_i + 1)],
                        wo_sb[l][0:32, E * 2 * p:E * (2 * p + 1)],
                        start=(p == 0), stop=False,
                        tile_position=(0, 0))
                    nc.tensor.matmul(
                        wp2[:, 0:E],
                        otp[p][64:96, 128 * qt_i:128 * (qt_i + 1)],
                        wo_sb[l][64:96, E * (2 * p + 1):E * (2 * p + 2)],
                        start=(p == 0), stop=(p == 3),
                        tile_position=(64, 0))
                nc.tensor.matmul(wp[:, 0:E], ones_row[0:1, 0:128],
                                 wob_sb[l], start=False, stop=True,
                                 tile_position=(0, 0))
                sl = slice(E * qt_i, E * (qt_i + 1))
                nc.vector.tensor_tensor(out=h[0:mw, sl], in0=h[0:mw, sl],
                                        in1=wp[0:mw, 0:E], op=OP.add)
                nc.vector.tensor_tensor(out=h[0:mw, sl], in0=h[0:mw, sl],
                                        in1=wp2[0:mw, 0:E], op=OP.add)

            # ---- MLP ----
            layer_norm(ln2s_sb[l], ln2b_sb[l], ytA, ytB)
            for f in range(8):
                for c0, cw in QC_PROJ:
                    gp = big.tile([128, 1536], F32, tag="big")
                    for t in range(2):
                        nc.tensor.matmul(
                            gp[:, 0:cw],
                            w1_sb[l][:, FF * t + 128 * f:FF * t + 128 * f + 128],
                            (ytA if t == 0 else ytB)[:, c0:c0 + cw],
                            start=(t == 0), stop=(t == 1))
                    nc.scalar.activation(gt[:, SP * f + c0:SP * f + c0 + cw],
                                         gp[:, 0:cw], AF.Gelu,
                                         bias=b1f_sb[l][:, f:f + 1], scale=1.0)
            for qt_i in range(NQ):
                mw = 128 if qt_i < 8 else 1
                wp = big.tile([128, 1536], F32, tag="big")
                for f in range(8):
                    nc.tensor.matmul(wp[:, 0:E],
                                     gt[:, SP * f + 128 * qt_i:
                                        SP * f + 128 * (qt_i + 1)],
                                     w2_sb[l][:, E * f:E * (f + 1)],
                                     start=(f == 0), stop=False)
                nc.tensor.matmul(wp[:, 0:E], ones_row[0:1, 0:128],
                                 b2r_sb[l], start=False, stop=True)
                sl = slice(E * qt_i, E * (qt_i + 1))
                nc.vector.tensor_tensor(out=h[0:mw, sl], in0=h[0:mw, sl],
                                        in1=wp[0:mw, 0:E], op=OP.add)
            if DBG == 1:
                nc.sync.dma_start(dbg_d[:, NQ * E * l:NQ * E * (l + 1)], h[:])

        # ---------------- classifier ----------------
        hbf = ap_.tile([128, NQ * E], BF16, tag="hbf")
        nc.vector.tensor_copy(hbf[:], h[:])
        mp = big.tile([128, 1536], F32, tag="big")
        for j in range(NQ):
            nc.tensor.matmul(mp[0:1, 0:E], ones_col[:],
                             hbf[:, E * j:E * (j + 1)],
                             start=(j == 0), stop=(j == NQ - 1))
        pbf = dnp.tile([1, E], BF16, tag="pbf")
        nc.scalar.activation(pbf[:], mp[0:1, 0:E], AF.Identity, bias=0.0,
                             scale=1.0 / S)
        ptp_ps = big.tile([128, 1536], BF16, tag="big")
        pT = dnp.tile([128, 2], BF16, tag="pT")
        for t in range(2):
            nc.tensor.transpose(ptp_ps[:, 128 * t:128 * t + 1],
                                pbf[0:1, 128 * t:128 * (t + 1)],
                                ident[0:1, 0:1])
            nc.vector.tensor_copy(pT[:, t:t + 1],
                                  ptp_ps[:, 128 * t:128 * t + 1])
        c1p = big.tile([128, 1536], F32, tag="big")
        for t in range(2):
            nc.tensor.matmul(c1p[0:1, 0:E], pT[:, t:t + 1],
                             wc1_sb[:, E * t:E * (t + 1)],
                             start=(t == 0), stop=False)
        nc.tensor.matmul(c1p[0:1, 0:E], ones_row[0:1, 0:1], bc1_sb,
                         start=False, stop=True)
        p2 = dnp.tile([1, E], F32, tag="p2")
        nc.vector.tensor_copy(p2[:], c1p[0:1, 0:E])
        st1 = dnp.tile([1, 6], F32, tag="st1")
        ag1 = dnp.tile([1, 2], F32, tag="ag1")
        nc.vector.bn_stats(st1[:], p2[:])
        nc.vector.bn_aggr(ag1[:], st1[:])
        r1 = dnp.tile([1, 2], F32, tag="r1")
        nc.vector.tensor_scalar_add(r1[:, 0:1], ag1[:, 1:2], EPS)
        nc.vector.reciprocal_approx_fast(r1[:, 0:1], r1[:, 0:1])
        nc.scalar.activation(r1[:, 0:1], r1[:, 0:1], AF.Sqrt)
        nc.vector.tensor_scalar(out=p2[:], in0=p2[:], scalar1=ag1[:, 0:1],
                                scalar2=r1[:, 0:1], op0=OP.subtract,
                                op1=OP.mult)
        nc.vector.tensor_tensor(out=p2[:], in0=p2[:], in1=lncg_sb,
                                op=OP.mult)
        nc.vector.tensor_tensor(out=p2[:], in0=p2[:], in1=lncb_sb,
                                op=OP.add)
        p2b = dnp.tile([1, E], BF16, tag="p2b")
        nc.vector.tensor_copy(p2b[:], p2[:])
        p2T = dnp.tile([128, 2], BF16, tag="p2T")
        for t in range(2):
            tp2 = big.tile([128, 1536], BF16, tag="big")
            nc.tensor.transpose(tp2[:, 0:1], p2b[0:1, 128 * t:128 * (t + 1)],
                                ident[0:1, 0:1])
            nc.vector.tensor_copy(p2T[:, t:t + 1], tp2[:, 0:1])
        op_ = big.tile([128, 1536], F32, tag="big")
        for t in range(2):
            nc.tensor.matmul(op_[0:1, 0:NCLS], p2T[:, t:t + 1],
                             wc2_sb[:, NCLS * t:NCLS * (t + 1)],
                             start=(t == 0), stop=False)
        nc.tensor.matmul(op_[0:1, 0:NCLS], ones_row[0:1, 0:1], bc2_sb,
                         start=False, stop=True)
        osb = dnp.tile([1, NCLS], F32, tag="osb")
        nc.vector.tensor_copy(osb[:], op_[0:1, 0:NCLS])
        nc.sync.dma_start(out_d[:], osb[:])

        for _p in (pvp, big, dnp, bcp, ptp, ap_, cp):
            _p.release()

    nc.compile()
    return nc


def _prep_shared(inputs):
    """Host-side weight preparation (shared across cores)."""
    bf16 = ml_dtypes.bfloat16
    f32 = np.float32
    g = {k: np.asarray(v, dtype=f32) for k, v in inputs.items()}
    d = {}

    perm = np.concatenate([np.arange(32) + 32 * SIG[s] for s in range(8)])

    def pack_k(w):  # [256, X] -> [128, 2X]
        return np.concatenate([w[0:128], w[128:256]], axis=1)

    # ---- embw blob ----
    embw = np.zeros((128, EMBW_COLS), f32)
    embw[0:C, OE_WEMB:OE_WEMB + E] = g["W_emb"]
    embw[0:1, OE_BEMB:OE_BEMB + E] = g["b_emb"].reshape(1, E)
    embw[:, OE_WC1:OE_WC1 + 2 * E] = pack_k(g["Wc1"])
    embw[0:1, OE_BC1:OE_BC1 + E] = g["bc1"].reshape(1, E)
    embw[:, OE_WC2:OE_WC2 + 2 * NCLS] = pack_k(g["Wc2"])
    embw[0:1, OE_BC2:OE_BC2 + NCLS] = g["bc2"].reshape(1, NCLS)
    d["embw"] = embw.astype(bf16)

    # ---- cb blob (fp32) ----
    cbb = np.zeros((128, CB_COLS), f32)
    for j in range(NQ):
        n = 128 if j < 8 else 1
        cbb[0:n, OC_POS + E * j:OC_POS + E * (j + 1)] = \
            g["pos"][128 * j:128 * j + n]
    cbb[:, OC_GEMB:OC_GEMB + E] = np.broadcast_to(g["g_emb"], (128, E))
    cbb[:, OC_BEEMB:OC_BEEMB + E] = np.broadcast_to(g["be_emb"], (128, E))
    cbb[0:1, OC_CLS:OC_CLS + E] = g["cls_token"].reshape(1, E)
    cbb[0:1, OC_LNCG:OC_LNCG + E] = g["lnc_g"].reshape(1, E)
    cbb[0:1, OC_LNCB:OC_LNCB + E] = g["lnc_b"].reshape(1, E)
    for l in range(L):
        o = OC_LAY + 20 * l
        cbb[:, o + 0:o + 2] = pack_k(g["ln1_g"][l].reshape(E, 1))
        cbb[:, o + 2:o + 4] = pack_k(g["ln1_b"][l].reshape(E, 1))
        cbb[:, o + 4:o + 6] = pack_k(g["ln2_g"][l].reshape(E, 1))
        cbb[:, o + 6:o + 8] = pack_k(g["ln2_b"][l].reshape(E, 1))
        cbb[:, o + 8:o + 10] = pack_k(g["bq"][l][perm].reshape(E, 1))
        cbb[:, o + 10:o + 12] = pack_k(g["bk"][l][perm].reshape(E, 1))
        cbb[:, o + 12:o + 20] = np.stack(
            [g["b1"][l][128 * t:128 * (t + 1)] for t in range(8)], axis=1)
    d["cb"] = cbb

    # ---- per-layer bf16 blobs ----
    for l in range(L):
        wb = np.zeros((128, WBL_COLS), f32)
        wb[:, OW_Q:OW_Q + 2 * E] = pack_k(g["Wq"][l][:, perm])
        wb[:, OW_K:OW_K + 2 * E] = pack_k(g["Wk"][l][:, perm])
        wb[:, OW_V:OW_V + 2 * E] = pack_k(g["Wv"][l][:, perm])
        for s in range(8):
            r0 = 0 if s % 2 == 0 else 64
            wb[r0:r0 + 32, OW_O + E * s:OW_O + E * (s + 1)] = \
                g["Wo"][l][32 * SIG[s]:32 * SIG[s] + 32]
        wb[:, OW_1:OW_1 + 2 * FF] = pack_k(g["W1"][l])
        wb[:, OW_2:OW_2 + 8 * E] = np.concatenate(
            [g["W2"][l][128 * t:128 * (t + 1)] for t in range(8)], axis=1)
        wb[0:1, OW_OB:OW_OB + E] = \
            (g["bo"][l] + g["bv"][l] @ g["Wo"][l]).reshape(1, E)
        wb[0:1, OW_B2:OW_B2 + E] = g["b2"][l].reshape(1, E)
        d[f"wbl{l}"] = wb.astype(bf16)
    return d


def kernel(**inputs):
    sys.path.insert(0, TRN_REPO)
    from concourse.bass_utils import run_bass_kernel_spmd

    if "nc" not in _CACHE:
        _CACHE["nc"] = _build()
    nc = _CACHE["nc"]

    shared = _prep_shared(inputs)
    x = np.asarray(inputs["x"], dtype=np.float32)
    in_maps = [dict(shared, x=np.ascontiguousarray(x[c])) for c in range(B)]
    res = run_bass_kernel_spmd(nc, in_maps, list(range(B)))
    out = np.stack([np.asarray(res.results[c]["out"]).reshape(NCLS)
                    for c in range(B)])
    return out.astype(np.float32)
